# revision 3
# baseline (speedup 1.0000x reference)
"""MinibatchDiscrimination kernel for 8 Trainium2 NeuronCores.

Reference computation (N=512, D=512, O=64, H=16):
    M   = einsum('nd,doh->noh', x, T)                  # [N, O, H]
    l1  = |M[i] - M[j]| summed over h                  # [N, N, O]
    out = exp(-l1).sum(axis=0) - 1                     # [N, O]
    ret = concat([x, out], axis=1)                     # [N, D+O]

Sharding: row-parallel over the batch dim. Core c receives x rolled by
-64*c rows, so every core runs the identical program computing rows 0:64
of its (rolled) batch against all 512 rows; host stacking of the 8 row
blocks reconstructs the full output in original order. No collectives.

Algebra: |d| = 2*max(d,0) - d telescopes over h, so
    l1[i,j,o] = 2*P_i[o,j] - Mo_sum[o,j] + Mo_sum[o,i],
    P_i[o,j]  = sum_h max(M_T[oh,j] - M_T[oh,i], 0).
The h-sum rides the PE as 0/1-mask matmuls; -Mo_sum/2 is injected into
the same PSUM accumulation by an identity matmul; the per-i +Mo_sum[:,i]
folds into the exp bias on ACT, whose free-dim accumulator performs the
j-sum. bf16 is safe: the self-term l1[i,i] is exactly 0 by construction
and every off-diagonal exp(-l1) underflows to dust.

v2 performance structure (the v1 limiter was the PE running its 18
matmuls per row-pair fully serial, ~3.9us/pair):
  * Mask matmuls use FOUR concurrent PE column-groups (tile_position
    (0,0)/(0,32)/(0,64)/(0,96), 32-wide masks): 4 MMs issue within
    ~10ns and retire as a quad every ~216ns -> 16 mask MMs in ~0.9us.
    PSUM partition layout stays p = 64*half + o.
  * DVE dual-op tensor_scalar runs at 4x-mode (~262ns/tile measured);
    split 13 tiles DVE / 3 tiles ACT Relu per pair to balance engines.
  * T is DMA'd in eight [D,128] oh-slices so M_T[t] production pipelines
    with the main loop: first DVE tile starts ~6us in, vs ~24us in v1.
  * Two rows per [128,512] PSUM tile; one exp+accumulate on ACT covers
    both rows; exp emission lags production (LAG=3).
"""
import numpy as np
import ml_dtypes

N, D, O, H = 512, 512, 64, 16
OH = O * H          # 1024
NCORES = 8
R = N // NCORES     # 64 rows per core
NT = OH // 128      # 8 oh-tiles of 128 partitions
ND = D // 128       # 4 contraction chunks
ACT_TILES = ((0, 7), (1, 6), (1, 7))  # (half, t) pairs handled by ACT Relu

_cache = {}


def _mask32_np():
    # mask32[p, t, m] = 1 where m = (o - 32*(t//4)) for the o covered by
    # partition p in oh-tile t (oh = o*16 + h; tile t covers o in
    # [8t, 8t+8)).  32-wide so four PE column-groups run concurrently.
    m = np.zeros((128, NT, 32), dtype=np.float32)
    p = np.arange(128)
    for t in range(NT):
        o = 8 * t + p // H
        m[p, t, o - 32 * (t // 4)] = 1.0
    return m.astype(ml_dtypes.bfloat16)


def _build():
    import concourse.bass as bass
    import concourse.tile as tile
    from concourse import bacc, mybir

    f32 = mybir.dt.float32
    bf16 = mybir.dt.bfloat16
    Alu = mybir.AluOpType
    Act = mybir.ActivationFunctionType

    nc = bacc.Bacc("TRN2", target_bir_lowering=False, debug=False,
                   enable_asserts=False, num_devices=NCORES)
    x_d = nc.dram_tensor("x", [N, D], f32, kind="ExternalInput").ap()
    t_d = nc.dram_tensor("T", [D, O, H], f32, kind="ExternalInput").ap()
    mask_d = nc.dram_tensor("mask", [128, NT, 32], bf16, kind="ExternalInput").ap()
    id_d = nc.dram_tensor("ident", [O, 2 * O], bf16, kind="ExternalInput").ap()
    id128_d = nc.dram_tensor("ident128", [128, 128], bf16, kind="ExternalInput").ap()
    out_d = nc.dram_tensor("out", [R, D + O], f32, kind="ExternalOutput").ap()
    # [D, OH] view, then sliced per 128-wide oh-slab for pipelined loads
    w_d = t_d.rearrange("d o h -> d (o h)").rearrange(
        "(dc p) oh -> p dc oh", p=128)  # [128, ND, OH]

    with tile.TileContext(nc) as tc:
        with (
            tc.tile_pool(name="const", bufs=1) as cpool,
            tc.tile_pool(name="stage", bufs=1) as spool,
            tc.tile_pool(name="a", bufs=6) as apool,
            tc.tile_pool(name="e", bufs=4) as epool,
            tc.tile_pool(name="mmps", bufs=2, space=bass.MemorySpace.PSUM) as mmps,
            tc.tile_pool(name="l1ps", bufs=4, space=bass.MemorySpace.PSUM) as l1ps,
        ):
            # ---- Stage A: constants + x ----------------------------------
            mask = cpool.tile([128, NT, 32], bf16)
            nc.sync.dma_start(mask[:], mask_d[:])
            ident = cpool.tile([O, 2 * O], bf16)  # [I64 | I64]
            nc.gpsimd.dma_start(ident[:], id_d[:])
            id128 = cpool.tile([128, 128], bf16)
            nc.gpsimd.dma_start(id128[:], id128_d[:])

            x_f = []
            x_b = []
            for nb in range(ND):
                xt = cpool.tile([128, D], f32, tag=f"x_f{nb}")
                xeng = nc.sync if nb % 2 == 0 else nc.scalar
                xeng.dma_start(xt[:], x_d[128 * nb:128 * (nb + 1), :])
                x_f.append(xt)
                xb = spool.tile([128, D], bf16, tag=f"x_b{nb}")
                nc.vector.tensor_copy(xb[:], xt[:])
                x_b.append(xb)

            # T in eight [D, 128] oh-slices so m_T[t] pipelines with use.
            w_sl = []
            for t in range(NT):
                wf = spool.tile([128, ND, 128], f32, tag=f"w_f{t}")
                weng = nc.gpsimd if t % 2 == 0 else nc.sync
                weng.dma_start(wf[:], w_d[:, :, 128 * t:128 * (t + 1)])
                wb = spool.tile([128, ND, 128], bf16, tag=f"w_b{t}")
                nc.vector.tensor_copy(wb[:], wf[:])
                w_sl.append(wb)

            # ---- Stage B: x^T (bf16) via PE transpose --------------------
            x_T = [spool.tile([128, N], bf16, tag=f"x_T{dc}", name=f"x_T{dc}")
                   for dc in range(ND)]
            for dc in range(ND):
                tp = mmps.tile([128, ND, 128], bf16, tag="tp")
                for nb in range(ND):
                    nc.tensor.transpose(tp[:, nb, :],
                                        x_b[nb][:, 128 * dc:128 * (dc + 1)],
                                        id128[:])
                if dc % 2 == 0:
                    nc.vector.tensor_copy(x_T[dc][:], tp.rearrange("p a b -> p (a b)")[:])
                else:
                    nc.scalar.copy(x_T[dc][:], tp.rearrange("p a b -> p (a b)")[:])

            # ---- Stage C: M_T = W^T @ x^T, per oh-slice ------------------
            m_T = []
            col_f = []  # my 64 bias columns, upcast bf16->f32 (exact)
            for t in range(NT):
                ps = mmps.tile([128, N], f32)
                for dc in range(ND):
                    nc.tensor.matmul(
                        ps[:],
                        w_sl[t][:, dc, :],
                        x_T[dc][:],
                        start=(dc == 0),
                        stop=(dc == ND - 1),
                    )
                mt = cpool.tile([128, N], bf16, tag=f"m_T{t}")
                if t % 2 == 0:
                    nc.vector.tensor_copy(mt[:], ps[:])
                else:
                    nc.scalar.copy(mt[:], ps[:])
                m_T.append(mt)
                cf = cpool.tile([128, R], f32, tag=f"col_f{t}")
                nc.scalar.copy(cf[:], mt[:, 0:R])
                col_f.append(cf)

            # col_neg[t] = -col_f[t] for the ACT Relu-bias tiles only
            col_n = {}
            for t in sorted({t for (_h, t) in ACT_TILES}):
                cn = cpool.tile([128, R], f32, tag=f"col_n{t}", name=f"col_n{t}")
                nc.scalar.mul(cn[:], col_f[t][:], -1.0)
                col_n[t] = cn

            # ---- Stage C2: Mo_sum[o, j] = sum_h M_T[oh, j] ---------------
            # Accumulate G = P + Mh (Mh = -Mo_sum/2 in bf16) on PE; fold the
            # per-i +Mo_sum[:, i] into the exp bias B = 2*Mh[:, i].
            mo_ps = mmps.tile([O, N], f32, tag="ps")
            for t in range(NT):
                lo = t // 4
                nc.tensor.matmul(mo_ps[32 * lo:32 * lo + 32, :],
                                 mask[:, t, :], m_T[t][:],
                                 start=(t % 4 == 0), stop=(t % 4 == 3),
                                 tile_position=(0, 32 * lo),
                                 skip_group_check=True)
            mh = cpool.tile([O, N], bf16)
            nc.vector.tensor_scalar_mul(mh[:], mo_ps[:], -0.5)
            # paired exp bias: rows 0:64 even-i cols, 64:128 odd-i cols
            bexp2 = cpool.tile([2 * O, R // 2], f32)
            nc.vector.tensor_scalar_mul(bexp2[0:O, :], mh[:, 0:R:2], 2.0)
            nc.vector.tensor_scalar_mul(bexp2[O:2 * O, :], mh[:, 1:R:2], 2.0)

            # ---- Stage D: main loop, two rows per PSUM tile --------------
            # Pair (2k, 2k+1): one [128, N] psum; even row in partitions
            # 0:64, odd in 64:128; p = 64*half + o.  13 of the 16 relu
            # tiles on DVE (dual-op tensor_scalar), 3 on ACT.  The two
            # ident MMs + 16 mask MMs are emitted in col-group quads so
            # four streams run concurrently on the PE.
            LAG = 3
            s_pair = cpool.tile([2 * O, R // 2], f32)
            pend = []

            def emit_exp(ps, k):
                e_scr = epool.tile([2 * O, N], bf16, tag="e_scr")
                nc.scalar.activation(
                    e_scr[:], ps[:], Act.Exp, bias=bexp2[:, k:k + 1], scale=-2.0,
                    accum_out=s_pair[:, k:k + 1],
                )

            act_set = set(ACT_TILES)
            for k in range(R // 2):
                abigs = []
                for half in range(2):
                    i = 2 * k + half
                    a_big = apool.tile([128, NT, N], bf16, tag="a_big")
                    for t in range(NT):
                        if (half, t) in act_set:
                            nc.scalar.activation(
                                a_big[:, t, :], m_T[t][:], Act.Relu,
                                bias=col_n[t][:, i:i + 1], scale=1.0,
                            )
                        else:
                            nc.vector.tensor_scalar(
                                a_big[:, t, :], m_T[t][:],
                                col_f[t][:, i:i + 1], 0.0,
                                Alu.subtract, Alu.max,
                            )
                    abigs.append(a_big)
                ps = l1ps.tile([2 * O, N], f32, tag="l1")
                for half in range(2):
                    nc.tensor.matmul(
                        ps[half * O:(half + 1) * O, :], ident[:, 0:O], mh[:],
                        start=True, stop=False, tile_position=(0, half * O),
                        skip_group_check=True,
                    )
                for q in range(4):
                    for half in range(2):
                        for lo in range(2):
                            t = q + 4 * lo
                            base = 64 * half + 32 * lo
                            nc.tensor.matmul(
                                ps[base:base + 32, :], mask[:, t, :],
                                abigs[half][:, t, :],
                                start=False, stop=(q == 3),
                                tile_position=(0, base),
                                skip_group_check=True,
                            )
                pend.append((ps, k))
                if len(pend) > LAG:
                    emit_exp(*pend.pop(0))
            for args in pend:
                emit_exp(*args)
            # unpack pairs into S[o, i]
            s_all = cpool.tile([O, R], f32)
            nc.vector.tensor_copy(s_all[:, 0:R:2], s_pair[0:O, :])
            nc.vector.tensor_copy(s_all[:, 1:R:2], s_pair[O:2 * O, :])

            # ---- Stage E: transpose S, subtract 1, write out -------------
            s_T = cpool.tile([R, O], f32)
            for a in range(2):
                for b in range(2):
                    nc.vector.transpose(
                        s_T[32 * a:32 * a + 32, 32 * b:32 * b + 32],
                        s_all[32 * b:32 * b + 32, 32 * a:32 * a + 32],
                    )
            o_small = cpool.tile([R, O], f32)
            nc.vector.tensor_scalar_add(o_small[:], s_T[:], -1.0)
            nc.sync.dma_start(out_d[:, D:D + O], o_small[:])
            nc.sync.dma_start(out_d[:, 0:D], x_f[0][0:R, :])

    nc.compile()
    return nc


def _get_nc():
    if "nc" not in _cache:
        _cache["nc"] = _build()
    return _cache["nc"]


def kernel(x, T):
    from concourse import bass_utils

    nc = _get_nc()
    x = np.ascontiguousarray(x, dtype=np.float32)
    T = np.ascontiguousarray(T, dtype=np.float32)
    mask = _mask32_np()
    ident = np.concatenate([np.eye(O), np.eye(O)], axis=1).astype(ml_dtypes.bfloat16)
    ident128 = np.eye(128, dtype=ml_dtypes.bfloat16)
    in_maps = [
        {"x": np.roll(x, -R * c, axis=0), "T": T, "mask": mask, "ident": ident, "ident128": ident128}
        for c in range(NCORES)
    ]
    res = bass_utils.run_bass_kernel_spmd(nc, in_maps, list(range(NCORES)))
    return np.concatenate([res.results[c]["out"] for c in range(NCORES)], axis=0)


# revision 7
# speedup vs baseline: 1.0391x; 1.0391x over previous
"""MinibatchDiscrimination kernel for 8 Trainium2 NeuronCores.

Reference computation (N=512, D=512, O=64, H=16):
    M   = einsum('nd,doh->noh', x, T)                  # [N, O, H]
    l1  = |M[i] - M[j]| summed over h                  # [N, N, O]
    out = exp(-l1).sum(axis=0) - 1                     # [N, O]
    ret = concat([x, out], axis=1)                     # [N, D+O]

Sharding: row-parallel over the batch dim. Core c receives x rolled by
-64*c rows, so every core runs the identical program computing rows 0:64
of its (rolled) batch against all 512 rows; host stacking of the 8 row
blocks reconstructs the full output in original order. No collectives.

Algebra: |d| = 2*max(d,0) - d telescopes over h, so
    l1[i,j,o] = 2*P_i[o,j] - Mo_sum[o,j] + Mo_sum[o,i],
    P_i[o,j]  = sum_h max(M_T[oh,j] - M_T[oh,i], 0).
The h-sum rides the PE as 0/1-mask matmuls; -Mo_sum/2 is injected into
the same PSUM accumulation by an identity matmul; the per-i +Mo_sum[:,i]
folds into the exp bias on ACT, whose free-dim accumulator performs the
j-sum. bf16 is safe: the self-term l1[i,i] is exactly 0 by construction
and every off-diagonal exp(-l1) underflows to dust.

v2 performance structure (the v1 limiter was the PE running its 18
matmuls per row-pair fully serial, ~3.9us/pair):
  * Mask matmuls use FOUR concurrent PE column-groups (tile_position
    (0,0)/(0,32)/(0,64)/(0,96), 32-wide masks): 4 MMs issue within
    ~10ns and retire as a quad every ~216ns -> 16 mask MMs in ~0.9us.
    PSUM partition layout stays p = 64*half + o.
  * DVE dual-op tensor_scalar runs at 4x-mode (~262ns/tile measured);
    split 13 tiles DVE / 3 tiles ACT Relu per pair to balance engines.
  * T is DMA'd in eight [D,128] oh-slices so M_T[t] production pipelines
    with the main loop: first DVE tile starts ~6us in, vs ~24us in v1.
  * Two rows per [128,512] PSUM tile; one exp+accumulate on ACT covers
    both rows; exp emission lags production (LAG=3).
"""
import numpy as np
import ml_dtypes

N, D, O, H = 512, 512, 64, 16
OH = O * H          # 1024
NCORES = 8
R = N // NCORES     # 64 rows per core
NT = OH // 128      # 8 oh-tiles of 128 partitions
ND = D // 128       # 4 contraction chunks
ACT_TILES = ((0, 7), (1, 6), (1, 7))  # (half, t) pairs handled by ACT Relu

_cache = {}


def _mask32_np():
    # mask32[p, t, m] = 1 where m = (o - 32*(t//4)) for the o covered by
    # partition p in oh-tile t (oh = o*16 + h; tile t covers o in
    # [8t, 8t+8)).  32-wide so four PE column-groups run concurrently.
    m = np.zeros((128, NT, 32), dtype=np.float32)
    p = np.arange(128)
    for t in range(NT):
        o = 8 * t + p // H
        m[p, t, o - 32 * (t // 4)] = 1.0
    return m.astype(ml_dtypes.bfloat16)


def _build():
    import concourse.bass as bass
    import concourse.tile as tile
    from concourse import bacc, mybir

    f32 = mybir.dt.float32
    bf16 = mybir.dt.bfloat16
    Alu = mybir.AluOpType
    Act = mybir.ActivationFunctionType

    nc = bacc.Bacc("TRN2", target_bir_lowering=False, debug=False,
                   enable_asserts=False, num_devices=NCORES)
    x_d = nc.dram_tensor("x", [N, D], f32, kind="ExternalInput").ap()
    t_d = nc.dram_tensor("T", [D, O, H], f32, kind="ExternalInput").ap()
    mask_d = nc.dram_tensor("mask", [128, NT, 32], bf16, kind="ExternalInput").ap()
    id_d = nc.dram_tensor("ident", [O, 2 * O], bf16, kind="ExternalInput").ap()
    id128_d = nc.dram_tensor("ident128", [128, 128], bf16, kind="ExternalInput").ap()
    out_d = nc.dram_tensor("out", [R, D + O], f32, kind="ExternalOutput").ap()
    # [D, OH] view, then sliced per 128-wide oh-slab for pipelined loads
    w_d = t_d.rearrange("d o h -> d (o h)").rearrange(
        "(dc p) oh -> p dc oh", p=128)  # [128, ND, OH]

    with tile.TileContext(nc) as tc:
        with (
            tc.tile_pool(name="const", bufs=1) as cpool,
            tc.tile_pool(name="stage", bufs=1) as spool,
            tc.tile_pool(name="a", bufs=6) as apool,
            tc.tile_pool(name="e", bufs=4) as epool,
            tc.tile_pool(name="mmps", bufs=2, space=bass.MemorySpace.PSUM) as mmps,
            tc.tile_pool(name="l1ps", bufs=4, space=bass.MemorySpace.PSUM) as l1ps,
        ):
            # ---- Stage A: x first, then T in four oh-slices, consts on
            # the idle vector/gpsimd queues.  DMA issue ops cost ~650ns
            # on the issuing engine, so keep the count low and the
            # ordering arrival-critical-first.
            x4_f = cpool.tile([128, ND, D], f32)   # x rows (p, nb) = x[128*nb+p]
            nc.sync.dma_start(
                x4_f[:], x_d.rearrange("(nb p) d -> p nb d", p=128))
            w_sl = []  # w_sl[g][:, dc, :] covers oh-slab pair (2g, 2g+1)
            for g in range(ND):
                wf = spool.tile([128, ND, 256], f32, tag=f"w_f{g}")
                nc.gpsimd.dma_start(wf[:], w_d[:, :, 256 * g:256 * (g + 1)])
                w_sl.append(wf)
            id128 = cpool.tile([128, 128], bf16)
            nc.scalar.dma_start(id128[:], id128_d[:])
            mask = cpool.tile([128, NT, 32], bf16)
            nc.scalar.dma_start(mask[:], mask_d[:])
            ident = cpool.tile([O, 2 * O], bf16)  # [I64 | I64]
            nc.scalar.dma_start(ident[:], id_d[:])

            x_b = spool.tile([128, ND, D], bf16)
            for g in range(2):
                nc.vector.tensor_copy(x_b[:, 2 * g:2 * g + 2, :],
                                      x4_f[:, 2 * g:2 * g + 2, :])
            w_b = []
            for g in range(ND):
                wb = spool.tile([128, ND, 256], bf16, tag=f"w_b{g}")
                nc.scalar.copy(wb[:], w_sl[g][:])
                w_b.append(wb)

            # ---- Stage B: x^T (bf16) via PE transpose --------------------
            x_T = [spool.tile([128, N], bf16, tag=f"x_T{dc}", name=f"x_T{dc}")
                   for dc in range(ND)]
            for dc in range(ND):
                tp = mmps.tile([128, ND, 128], bf16, tag="tp")
                for nb in range(ND):
                    nc.tensor.transpose(tp[:, nb, :],
                                        x_b[:, nb, 128 * dc:128 * (dc + 1)],
                                        id128[:])
                if dc % 2 == 0:
                    nc.vector.tensor_copy(x_T[dc][:], tp.rearrange("p a b -> p (a b)")[:])
                else:
                    nc.scalar.copy(x_T[dc][:], tp.rearrange("p a b -> p (a b)")[:])

            # ---- Stage C: M_T = W^T @ x^T, per oh-slice ------------------
            m_T = []
            col_f = []  # my 64 bias columns, upcast bf16->f32 (exact)
            for t in range(NT):
                ps = mmps.tile([128, N], f32)
                for dc in range(ND):
                    nc.tensor.matmul(
                        ps[:],
                        w_b[t // 2][:, dc, 128 * (t % 2):128 * (t % 2) + 128],
                        x_T[dc][:],
                        start=(dc == 0),
                        stop=(dc == ND - 1),
                    )
                mt = cpool.tile([128, N], bf16, tag=f"m_T{t}")
                if t < 4:
                    nc.vector.tensor_copy(mt[:], ps[:])
                else:
                    nc.scalar.copy(mt[:], ps[:])
                m_T.append(mt)
                cf = cpool.tile([128, R], f32, tag=f"col_f{t}")
                nc.scalar.copy(cf[:], mt[:, 0:R])
                col_f.append(cf)

            # col_neg[t] = -col_f[t] for the ACT Relu-bias tiles only
            col_n = {}
            for t in sorted({t for (_h, t) in ACT_TILES}):
                cn = cpool.tile([128, R], f32, tag=f"col_n{t}", name=f"col_n{t}")
                nc.scalar.mul(cn[:], col_f[t][:], -1.0)
                col_n[t] = cn

            # ---- Stage C2: Mo_sum[o, j] = sum_h M_T[oh, j] ---------------
            # Accumulate G = P + Mh (Mh = -Mo_sum/2 in bf16) on PE; fold the
            # per-i +Mo_sum[:, i] into the exp bias B = 2*Mh[:, i].
            mo_ps = mmps.tile([O, N], f32, tag="ps")
            for t in range(NT):
                lo = t // 4
                nc.tensor.matmul(mo_ps[32 * lo:32 * lo + 32, :],
                                 mask[:, t, :], m_T[t][:],
                                 start=(t % 4 == 0), stop=(t % 4 == 3),
                                 tile_position=(0, 32 * lo),
                                 skip_group_check=True)
            mh = cpool.tile([O, N], bf16)
            nc.scalar.mul(mh[:], mo_ps[:], -0.5)
            # paired exp bias: rows 0:64 even-i cols, 64:128 odd-i cols
            bexp2 = cpool.tile([2 * O, R // 2], f32)
            nc.scalar.mul(bexp2[0:O, :], mh[:, 0:R:2], 2.0)
            nc.scalar.mul(bexp2[O:2 * O, :], mh[:, 1:R:2], 2.0)

            # ---- Stage D: main loop, two rows per PSUM tile --------------
            # Pair (2k, 2k+1): one [128, N] psum; even row in partitions
            # 0:64, odd in 64:128; p = 64*half + o.  13 of the 16 relu
            # tiles on DVE (dual-op tensor_scalar), 3 on ACT.  The two
            # ident MMs + 16 mask MMs are emitted in col-group quads so
            # four streams run concurrently on the PE.
            LAG = 3
            s_pair = cpool.tile([2 * O, R // 2], f32)
            pend = []

            def emit_exp(ps, k):
                e_scr = epool.tile([2 * O, N], bf16, tag="e_scr")
                nc.scalar.activation(
                    e_scr[:], ps[:], Act.Exp, bias=bexp2[:, k:k + 1], scale=-2.0,
                    accum_out=s_pair[:, k:k + 1],
                )

            act_set = set(ACT_TILES)
            for k in range(R // 2):
                abigs = []
                for half in range(2):
                    i = 2 * k + half
                    a_big = apool.tile([128, NT, N], bf16, tag="a_big")
                    for t in range(NT):
                        if (half, t) in act_set:
                            nc.scalar.activation(
                                a_big[:, t, :], m_T[t][:], Act.Relu,
                                bias=col_n[t][:, i:i + 1], scale=1.0,
                            )
                        else:
                            nc.vector.tensor_scalar(
                                a_big[:, t, :], m_T[t][:],
                                col_f[t][:, i:i + 1], 0.0,
                                Alu.subtract, Alu.max,
                            )
                    abigs.append(a_big)
                ps = l1ps.tile([2 * O, N], f32, tag="l1")
                for half in range(2):
                    nc.tensor.matmul(
                        ps[half * O:(half + 1) * O, :], ident[:, 0:O], mh[:],
                        start=True, stop=False, tile_position=(0, half * O),
                        skip_group_check=True,
                    )
                for q in range(4):
                    for half in range(2):
                        for lo in range(2):
                            t = q + 4 * lo
                            base = 64 * half + 32 * lo
                            nc.tensor.matmul(
                                ps[base:base + 32, :], mask[:, t, :],
                                abigs[half][:, t, :],
                                start=False, stop=(q == 3),
                                tile_position=(0, base),
                                skip_group_check=True,
                            )
                pend.append((ps, k))
                if len(pend) > LAG:
                    emit_exp(*pend.pop(0))
            for args in pend:
                emit_exp(*args)
            # unpack pairs into S[o, i]
            s_all = cpool.tile([O, R], f32)
            nc.vector.tensor_copy(s_all[:, 0:R:2], s_pair[0:O, :])
            nc.vector.tensor_copy(s_all[:, 1:R:2], s_pair[O:2 * O, :])

            # ---- Stage E: transpose S, subtract 1, write out -------------
            s_T = cpool.tile([R, O], f32)
            for a in range(2):
                for b in range(2):
                    nc.vector.transpose(
                        s_T[32 * a:32 * a + 32, 32 * b:32 * b + 32],
                        s_all[32 * b:32 * b + 32, 32 * a:32 * a + 32],
                    )
            o_small = cpool.tile([R, O], f32)
            nc.vector.tensor_scalar_add(o_small[:], s_T[:], -1.0)
            nc.sync.dma_start(out_d[:, D:D + O], o_small[:])
            nc.sync.dma_start(out_d[:, 0:D], x4_f[0:R, 0, :])

    nc.compile()
    return nc


def _get_nc():
    if "nc" not in _cache:
        _cache["nc"] = _build()
    return _cache["nc"]


def kernel(x, T):
    from concourse import bass_utils

    nc = _get_nc()
    x = np.ascontiguousarray(x, dtype=np.float32)
    T = np.ascontiguousarray(T, dtype=np.float32)
    mask = _mask32_np()
    ident = np.concatenate([np.eye(O), np.eye(O)], axis=1).astype(ml_dtypes.bfloat16)
    ident128 = np.eye(128, dtype=ml_dtypes.bfloat16)
    in_maps = [
        {"x": np.roll(x, -R * c, axis=0), "T": T, "mask": mask, "ident": ident, "ident128": ident128}
        for c in range(NCORES)
    ]
    res = bass_utils.run_bass_kernel_spmd(nc, in_maps, list(range(NCORES)))
    return np.concatenate([res.results[c]["out"] for c in range(NCORES)], axis=0)


# revision 10
# speedup vs baseline: 1.0419x; 1.0026x over previous
"""MinibatchDiscrimination kernel for 8 Trainium2 NeuronCores.

Reference computation (N=512, D=512, O=64, H=16):
    M   = einsum('nd,doh->noh', x, T)                  # [N, O, H]
    l1  = |M[i] - M[j]| summed over h                  # [N, N, O]
    out = exp(-l1).sum(axis=0) - 1                     # [N, O]
    ret = concat([x, out], axis=1)                     # [N, D+O]

Sharding: row-parallel over the batch dim. Core c receives x rolled by
-64*c rows, so every core runs the identical program computing rows 0:64
of its (rolled) batch against all 512 rows; host stacking of the 8 row
blocks reconstructs the full output in original order. No collectives.

Algebra: |d| = 2*max(d,0) - d telescopes over h, so
    l1[i,j,o] = 2*P_i[o,j] - Mo_sum[o,j] + Mo_sum[o,i],
    P_i[o,j]  = sum_h max(M_T[oh,j] - M_T[oh,i], 0).
The h-sum rides the PE as 0/1-mask matmuls; -Mo_sum/2 is injected into
the same PSUM accumulation by an identity matmul; the per-i +Mo_sum[:,i]
folds into the exp bias on ACT, whose free-dim accumulator performs the
j-sum. bf16 is safe: the self-term l1[i,i] is exactly 0 by construction
and every off-diagonal exp(-l1) underflows to dust.

v2 performance structure (the v1 limiter was the PE running its 18
matmuls per row-pair fully serial, ~3.9us/pair):
  * Mask matmuls use FOUR concurrent PE column-groups (tile_position
    (0,0)/(0,32)/(0,64)/(0,96), 32-wide masks): 4 MMs issue within
    ~10ns and retire as a quad every ~216ns -> 16 mask MMs in ~0.9us.
    PSUM partition layout stays p = 64*half + o.
  * DVE dual-op tensor_scalar runs at 4x-mode (~263ns/tile measured);
    the 16 relu tiles per pair split 13/3 DVE/ACT on even pairs and
    12/4 on odd pairs (12.5 avg) to balance the two engines.
  * T is DMA'd in four [D,256] oh-slice pairs so M_T[t] production
    pipelines with the main loop; junk warm-up ops burn the DMA wait
    so engines exit their cold-clock state before the real pipeline.
  * Two rows per [128,512] PSUM tile; one exp+accumulate on ACT covers
    both rows; exp emission lags production (LAG=3).
"""
import numpy as np
import ml_dtypes

N, D, O, H = 512, 512, 64, 16
OH = O * H          # 1024
NCORES = 8
R = N // NCORES     # 64 rows per core
NT = OH // 128      # 8 oh-tiles of 128 partitions
ND = D // 128       # 4 contraction chunks
ACT_SPLIT = (((0, 7), (1, 6), (1, 7)),                 # even pairs: 13 DVE / 3 ACT
             ((0, 6), (0, 7), (1, 6), (1, 7)))        # odd pairs: 12 DVE / 4 ACT

_cache = {}


def _mask32_np():
    # mask32[p, t, m] = 1 where m = (o - 32*(t//4)) for the o covered by
    # partition p in oh-tile t (oh = o*16 + h; tile t covers o in
    # [8t, 8t+8)).  32-wide so four PE column-groups run concurrently.
    m = np.zeros((128, NT, 32), dtype=np.float32)
    p = np.arange(128)
    for t in range(NT):
        o = 8 * t + p // H
        m[p, t, o - 32 * (t // 4)] = 1.0
    return m.astype(ml_dtypes.bfloat16)


def _build():
    import concourse.bass as bass
    import concourse.tile as tile
    from concourse import bacc, mybir

    f32 = mybir.dt.float32
    bf16 = mybir.dt.bfloat16
    Alu = mybir.AluOpType
    Act = mybir.ActivationFunctionType

    nc = bacc.Bacc("TRN2", target_bir_lowering=False, debug=False,
                   enable_asserts=False, num_devices=NCORES)
    x_d = nc.dram_tensor("x", [N, D], f32, kind="ExternalInput").ap()
    t_d = nc.dram_tensor("T", [D, O, H], f32, kind="ExternalInput").ap()
    mask_d = nc.dram_tensor("mask", [128, NT, 32], bf16, kind="ExternalInput").ap()
    id_d = nc.dram_tensor("ident", [O, 2 * O], bf16, kind="ExternalInput").ap()
    id128_d = nc.dram_tensor("ident128", [128, 128], bf16, kind="ExternalInput").ap()
    out_d = nc.dram_tensor("out", [R, D + O], f32, kind="ExternalOutput").ap()
    # [D, OH] view, then sliced per 128-wide oh-slab for pipelined loads
    w_d = t_d.rearrange("d o h -> d (o h)").rearrange(
        "(dc p) oh -> p dc oh", p=128)  # [128, ND, OH]

    with tile.TileContext(nc) as tc:
        with (
            tc.tile_pool(name="const", bufs=1) as cpool,
            tc.tile_pool(name="stage", bufs=1) as spool,
            tc.tile_pool(name="a", bufs=6) as apool,
            tc.tile_pool(name="e", bufs=4) as epool,
            tc.tile_pool(name="mmps", bufs=2, space=bass.MemorySpace.PSUM) as mmps,
            tc.tile_pool(name="l1ps", bufs=4, space=bass.MemorySpace.PSUM) as l1ps,
        ):
            # ---- Stage A: x first, then T in four oh-slices, consts on
            # the idle vector/gpsimd queues.  DMA issue ops cost ~650ns
            # on the issuing engine, so keep the count low and the
            # ordering arrival-critical-first.
            # Warmup: engines start at their throttled clocks and take
            # ~3.4us of sustained activity to unthrottle.  Burn the DMA
            # wait on junk ops so the real pipeline starts warm.
            wz = cpool.tile([128, 512], bf16)
            nc.gpsimd.memset(wz[:], 0.0)
            j128 = cpool.tile([128, 128], bf16)
            nc.gpsimd.memset(j128[:], 0.0)
            wscr = cpool.tile([128, 512], bf16)
            wps = mmps.tile([128, N], f32, tag="tp", name="wps")
            for r in range(14):
                nc.tensor.matmul(wps[:], j128[:], wz[:],
                                 start=(r == 0), stop=(r == 13))
            for r in range(12):
                nc.vector.tensor_copy(wscr[:], wz[:])
            for r in range(3):
                nc.scalar.copy(wscr[:], wz[:])

            x_f = []
            x_b = []
            for nb in range(ND):
                xt = cpool.tile([128, D], f32, tag=f"x_f{nb}")
                xeng = nc.sync if nb % 2 == 0 else nc.scalar
                xeng.dma_start(xt[:], x_d[128 * nb:128 * (nb + 1), :])
                x_f.append(xt)
            w_sl = []  # w_sl[g][:, dc, :] covers oh-slab pair (2g, 2g+1)
            for g in range(ND):
                wf = spool.tile([128, ND, 256], f32, tag=f"w_f{g}")
                nc.gpsimd.dma_start(wf[:], w_d[:, :, 256 * g:256 * (g + 1)])
                w_sl.append(wf)
            id128 = cpool.tile([128, 128], bf16)
            nc.scalar.dma_start(id128[:], id128_d[:])
            mask = cpool.tile([128, NT, 32], bf16)
            nc.scalar.dma_start(mask[:], mask_d[:])
            ident = cpool.tile([O, 2 * O], bf16)  # [I64 | I64]
            nc.scalar.dma_start(ident[:], id_d[:])

            for nb in range(ND):
                xb = spool.tile([128, D], bf16, tag=f"x_b{nb}")
                nc.vector.tensor_copy(xb[:], x_f[nb][:])
                x_b.append(xb)
            w_b = []
            for g in range(ND):
                wb = spool.tile([128, ND, 256], bf16, tag=f"w_b{g}")
                nc.scalar.copy(wb[:], w_sl[g][:])
                w_b.append(wb)

            # ---- Stage B: x^T (bf16) via PE transpose --------------------
            x_T = [spool.tile([128, N], bf16, tag=f"x_T{dc}", name=f"x_T{dc}")
                   for dc in range(ND)]
            for dc in range(ND):
                tp = mmps.tile([128, ND, 128], bf16, tag="tp")
                for nb in range(ND):
                    nc.tensor.transpose(tp[:, nb, :],
                                        x_b[nb][:, 128 * dc:128 * (dc + 1)],
                                        id128[:])
                if dc % 2 == 0:
                    nc.vector.tensor_copy(x_T[dc][:], tp.rearrange("p a b -> p (a b)")[:])
                else:
                    nc.scalar.copy(x_T[dc][:], tp.rearrange("p a b -> p (a b)")[:])

            # ---- Stage C: M_T = W^T @ x^T, per oh-slice ------------------
            m_T = []
            col_f = []  # my 64 bias columns, upcast bf16->f32 (exact)
            for t in range(NT):
                ps = mmps.tile([128, N], f32)
                for dc in range(ND):
                    nc.tensor.matmul(
                        ps[:],
                        w_b[t // 2][:, dc, 128 * (t % 2):128 * (t % 2) + 128],
                        x_T[dc][:],
                        start=(dc == 0),
                        stop=(dc == ND - 1),
                    )
                mt = cpool.tile([128, N], bf16, tag=f"m_T{t}")
                if t < 4:
                    nc.vector.tensor_copy(mt[:], ps[:])
                else:
                    nc.scalar.copy(mt[:], ps[:])
                m_T.append(mt)
                cf = cpool.tile([128, R], f32, tag=f"col_f{t}")
                nc.scalar.copy(cf[:], mt[:, 0:R])
                col_f.append(cf)

            # col_neg[t] = -col_f[t] for the ACT Relu-bias tiles only
            col_n = {}
            for t in sorted({t for par in ACT_SPLIT for (_h, t) in par}):
                cn = cpool.tile([128, R], f32, tag=f"col_n{t}", name=f"col_n{t}")
                nc.scalar.mul(cn[:], col_f[t][:], -1.0)
                col_n[t] = cn

            # ---- Stage C2: Mo_sum[o, j] = sum_h M_T[oh, j] ---------------
            # Accumulate G = P + Mh (Mh = -Mo_sum/2 in bf16) on PE; fold the
            # per-i +Mo_sum[:, i] into the exp bias B = 2*Mh[:, i].
            mo_ps = mmps.tile([O, N], f32, tag="ps")
            for t in range(NT):
                lo = t // 4
                nc.tensor.matmul(mo_ps[32 * lo:32 * lo + 32, :],
                                 mask[:, t, :], m_T[t][:],
                                 start=(t % 4 == 0), stop=(t % 4 == 3),
                                 tile_position=(0, 32 * lo),
                                 skip_group_check=True)
            mh = cpool.tile([O, N], bf16)
            nc.scalar.mul(mh[:], mo_ps[:], -0.5)
            # paired exp bias: rows 0:64 even-i cols, 64:128 odd-i cols
            bexp2 = cpool.tile([2 * O, R // 2], f32)
            nc.scalar.mul(bexp2[0:O, :], mh[:, 0:R:2], 2.0)
            nc.scalar.mul(bexp2[O:2 * O, :], mh[:, 1:R:2], 2.0)

            # ---- Stage D: main loop, two rows per PSUM tile --------------
            # Pair (2k, 2k+1): one [128, N] psum; even row in partitions
            # 0:64, odd in 64:128; p = 64*half + o.  13 of the 16 relu
            # tiles on DVE (dual-op tensor_scalar), 3 on ACT.  The two
            # ident MMs + 16 mask MMs are emitted in col-group quads so
            # four streams run concurrently on the PE.
            LAG = 3
            s_pair = cpool.tile([2 * O, R // 2], f32)
            pend = []

            def emit_exp(ps, k):
                e_scr = epool.tile([2 * O, N], bf16, tag="e_scr")
                nc.scalar.activation(
                    e_scr[:], ps[:], Act.Exp, bias=bexp2[:, k:k + 1], scale=-2.0,
                    accum_out=s_pair[:, k:k + 1],
                )

            for k in range(R // 2):
                act_set = set(ACT_SPLIT[k % 2])
                abigs = []
                for half in range(2):
                    i = 2 * k + half
                    a_big = apool.tile([128, NT, N], bf16, tag="a_big")
                    for t in range(NT):
                        if (half, t) in act_set:
                            nc.scalar.activation(
                                a_big[:, t, :], m_T[t][:], Act.Relu,
                                bias=col_n[t][:, i:i + 1], scale=1.0,
                            )
                        else:
                            nc.vector.tensor_scalar(
                                a_big[:, t, :], m_T[t][:],
                                col_f[t][:, i:i + 1], 0.0,
                                Alu.subtract, Alu.max,
                            )
                    abigs.append(a_big)
                ps = l1ps.tile([2 * O, N], f32, tag="l1")
                for half in range(2):
                    nc.tensor.matmul(
                        ps[half * O:(half + 1) * O, :], ident[:, 0:O], mh[:],
                        start=True, stop=False, tile_position=(0, half * O),
                        skip_group_check=True,
                    )
                for q in range(4):
                    for half in range(2):
                        for lo in range(2):
                            t = q + 4 * lo
                            base = 64 * half + 32 * lo
                            nc.tensor.matmul(
                                ps[base:base + 32, :], mask[:, t, :],
                                abigs[half][:, t, :],
                                start=False, stop=(q == 3),
                                tile_position=(0, base),
                                skip_group_check=True,
                            )
                pend.append((ps, k))
                if len(pend) > LAG:
                    emit_exp(*pend.pop(0))
            for args in pend:
                emit_exp(*args)
            # unpack pairs into S[o, i]
            s_all = cpool.tile([O, R], f32)
            nc.vector.tensor_copy(s_all[:, 0:R:2], s_pair[0:O, :])
            nc.vector.tensor_copy(s_all[:, 1:R:2], s_pair[O:2 * O, :])

            # ---- Stage E: transpose S, subtract 1, write out -------------
            s_T = cpool.tile([R, O], f32)
            for a in range(2):
                for b in range(2):
                    nc.vector.transpose(
                        s_T[32 * a:32 * a + 32, 32 * b:32 * b + 32],
                        s_all[32 * b:32 * b + 32, 32 * a:32 * a + 32],
                    )
            o_small = cpool.tile([R, O], f32)
            nc.vector.tensor_scalar_add(o_small[:], s_T[:], -1.0)
            nc.sync.dma_start(out_d[:, D:D + O], o_small[:])
            nc.sync.dma_start(out_d[:, 0:D], x_f[0][0:R, :])

    nc.compile()
    return nc


def _get_nc():
    if "nc" not in _cache:
        _cache["nc"] = _build()
    return _cache["nc"]


def kernel(x, T):
    from concourse import bass_utils

    nc = _get_nc()
    x = np.ascontiguousarray(x, dtype=np.float32)
    T = np.ascontiguousarray(T, dtype=np.float32)
    mask = _mask32_np()
    ident = np.concatenate([np.eye(O), np.eye(O)], axis=1).astype(ml_dtypes.bfloat16)
    ident128 = np.eye(128, dtype=ml_dtypes.bfloat16)
    in_maps = [
        {"x": np.roll(x, -R * c, axis=0), "T": T, "mask": mask, "ident": ident, "ident128": ident128}
        for c in range(NCORES)
    ]
    res = bass_utils.run_bass_kernel_spmd(nc, in_maps, list(range(NCORES)))
    return np.concatenate([res.results[c]["out"] for c in range(NCORES)], axis=0)


# revision 11
# speedup vs baseline: 1.0678x; 1.0249x over previous
"""MinibatchDiscrimination kernel for 8 Trainium2 NeuronCores.

Reference computation (N=512, D=512, O=64, H=16):
    M   = einsum('nd,doh->noh', x, T)                  # [N, O, H]
    l1  = |M[i] - M[j]| summed over h                  # [N, N, O]
    out = exp(-l1).sum(axis=0) - 1                     # [N, O]
    ret = concat([x, out], axis=1)                     # [N, D+O]

Sharding: row-parallel over the batch dim. Core c receives x rolled by
-64*c rows, so every core runs the identical program computing rows 0:64
of its (rolled) batch against all 512 rows; host stacking of the 8 row
blocks reconstructs the full output in original order. No collectives.

Algebra: |d| = 2*max(d,0) - d telescopes over h, so
    l1[i,j,o] = 2*P_i[o,j] - Mo_sum[o,j] + Mo_sum[o,i],
    P_i[o,j]  = sum_h max(M_T[oh,j] - M_T[oh,i], 0).
The h-sum rides the PE as 0/1-mask matmuls; -Mo_sum/2 is injected into
the same PSUM accumulation by an identity matmul; the per-i +Mo_sum[:,i]
folds into the exp bias on ACT, whose free-dim accumulator performs the
j-sum. bf16 is safe: the self-term l1[i,i] is exactly 0 by construction
and every off-diagonal exp(-l1) underflows to dust.

v2 performance structure (the v1 limiter was the PE running its 18
matmuls per row-pair fully serial, ~3.9us/pair):
  * Mask matmuls use FOUR concurrent PE column-groups (tile_position
    (0,0)/(0,32)/(0,64)/(0,96), 32-wide masks): 4 MMs issue within
    ~10ns and retire as a quad every ~216ns -> 16 mask MMs in ~0.9us.
    PSUM partition layout stays p = 64*half + o.
  * DVE dual-op tensor_scalar runs at 4x-mode (~263ns/tile measured);
    the 16 relu tiles per pair split 13/3 DVE/ACT on even pairs and
    12/4 on odd pairs (12.5 avg) to balance the two engines.
  * T is DMA'd in four [D,256] oh-slice pairs so M_T[t] production
    pipelines with the main loop; junk warm-up ops burn the DMA wait
    so engines exit their cold-clock state before the real pipeline.
  * Two rows per [128,512] PSUM tile; one exp+accumulate on ACT covers
    both rows; exp emission lags production (LAG=3).
"""
import numpy as np
import ml_dtypes

N, D, O, H = 512, 512, 64, 16
OH = O * H          # 1024
NCORES = 8
R = N // NCORES     # 64 rows per core
NT = OH // 128      # 8 oh-tiles of 128 partitions
ND = D // 128       # 4 contraction chunks
ACT_SPLIT = (((0, 7), (1, 6), (1, 7)),                 # even pairs: 13 DVE / 3 ACT
             ((0, 6), (0, 7), (1, 6), (1, 7)))        # odd pairs: 12 DVE / 4 ACT

_cache = {}


def _mask32_np():
    # mask32[p, t, m] = 1 where m = (o - 32*(t//4)) for the o covered by
    # partition p in oh-tile t (oh = o*16 + h; tile t covers o in
    # [8t, 8t+8)).  32-wide so four PE column-groups run concurrently.
    m = np.zeros((128, NT, 32), dtype=np.float32)
    p = np.arange(128)
    for t in range(NT):
        o = 8 * t + p // H
        m[p, t, o - 32 * (t // 4)] = 1.0
    return m.astype(ml_dtypes.bfloat16)


def _build():
    import concourse.bass as bass
    import concourse.tile as tile
    from concourse import bacc, mybir

    f32 = mybir.dt.float32
    bf16 = mybir.dt.bfloat16
    Alu = mybir.AluOpType
    Act = mybir.ActivationFunctionType

    nc = bacc.Bacc("TRN2", target_bir_lowering=False, debug=False,
                   enable_asserts=False, num_devices=NCORES)
    x_d = nc.dram_tensor("x", [N, D], f32, kind="ExternalInput").ap()
    t_d = nc.dram_tensor("T", [D, O, H], f32, kind="ExternalInput").ap()
    mask_d = nc.dram_tensor("mask", [128, NT, 32], bf16, kind="ExternalInput").ap()
    id_d = nc.dram_tensor("ident", [O, 2 * O], bf16, kind="ExternalInput").ap()
    id128_d = nc.dram_tensor("ident128", [128, 128], bf16, kind="ExternalInput").ap()
    out_d = nc.dram_tensor("out", [R, D + O], f32, kind="ExternalOutput").ap()
    # [D, OH] view, then sliced per 128-wide oh-slab for pipelined loads
    w_d = t_d.rearrange("d o h -> d (o h)").rearrange(
        "(dc p) oh -> p dc oh", p=128)  # [128, ND, OH]

    with tile.TileContext(nc) as tc:
        with (
            tc.tile_pool(name="const", bufs=1) as cpool,
            tc.tile_pool(name="stage", bufs=1) as spool,
            tc.tile_pool(name="a", bufs=6) as apool,
            tc.tile_pool(name="e", bufs=4) as epool,
            tc.tile_pool(name="mmps", bufs=2, space=bass.MemorySpace.PSUM) as mmps,
            tc.tile_pool(name="l1ps", bufs=4, space=bass.MemorySpace.PSUM) as l1ps,
        ):
            # ---- Stage A: x first, then T in four oh-slices, consts on
            # the idle vector/gpsimd queues.  DMA issue ops cost ~650ns
            # on the issuing engine, so keep the count low and the
            # ordering arrival-critical-first.
            # Warmup: engines start at their throttled clocks and take
            # ~3.4us of sustained activity to unthrottle.  Burn the DMA
            # wait on junk ops so the real pipeline starts warm.
            wz = cpool.tile([128, 512], bf16)
            nc.gpsimd.memset(wz[:], 0.0)
            j128 = cpool.tile([128, 128], bf16)
            nc.gpsimd.memset(j128[:], 0.0)
            wscr = cpool.tile([128, 512], bf16)
            wps = mmps.tile([128, N], f32, tag="tp", name="wps")
            for r in range(14):
                nc.tensor.matmul(wps[:], j128[:], wz[:],
                                 start=(r == 0), stop=(r == 13))
            for r in range(12):
                nc.vector.tensor_copy(wscr[:], wz[:])
            for r in range(3):
                nc.scalar.copy(wscr[:], wz[:])

            x_f = []
            x_b = []
            for nb in range(ND):
                xt = cpool.tile([128, D], f32, tag=f"x_f{nb}")
                xeng = nc.sync if nb % 2 == 0 else nc.scalar
                xeng.dma_start(xt[:], x_d[128 * nb:128 * (nb + 1), :])
                x_f.append(xt)
            w_sl = []  # w_sl[g][:, dc, :] covers oh-slab pair (2g, 2g+1)
            for g in range(ND):
                wf = spool.tile([128, ND, 256], f32, tag=f"w_f{g}")
                nc.gpsimd.dma_start(wf[:], w_d[:, :, 256 * g:256 * (g + 1)])
                w_sl.append(wf)
            id128 = cpool.tile([128, 128], bf16)
            nc.scalar.dma_start(id128[:], id128_d[:])
            mask = cpool.tile([128, NT, 32], bf16)
            nc.scalar.dma_start(mask[:], mask_d[:])
            ident = cpool.tile([O, 2 * O], bf16)  # [I64 | I64]
            nc.scalar.dma_start(ident[:], id_d[:])

            for nb in range(ND):
                xb = spool.tile([128, D], bf16, tag=f"x_b{nb}")
                nc.vector.tensor_copy(xb[:], x_f[nb][:])
                x_b.append(xb)
            w_b = []
            for g in range(ND):
                wb = spool.tile([128, ND, 256], bf16, tag=f"w_b{g}")
                nc.scalar.copy(wb[:], w_sl[g][:])
                w_b.append(wb)

            # ---- Stage B: x^T (bf16) via PE transpose --------------------
            x_T = [spool.tile([128, N], bf16, tag=f"x_T{dc}", name=f"x_T{dc}")
                   for dc in range(ND)]
            for dc in range(ND):
                tp = mmps.tile([128, ND, 128], bf16, tag="tp")
                for nb in range(ND):
                    nc.tensor.transpose(tp[:, nb, :],
                                        x_b[nb][:, 128 * dc:128 * (dc + 1)],
                                        id128[:])
                nc.vector.tensor_copy(x_T[dc][:], tp.rearrange("p a b -> p (a b)")[:])

            # ---- Stage C: M_T = W^T @ x^T, per oh-slice ------------------
            m_T = []
            col_f = []  # my 64 bias columns, upcast bf16->f32 (exact)
            for t in range(NT):
                ps = mmps.tile([128, N], f32)
                for dc in range(ND):
                    nc.tensor.matmul(
                        ps[:],
                        w_b[t // 2][:, dc, 128 * (t % 2):128 * (t % 2) + 128],
                        x_T[dc][:],
                        start=(dc == 0),
                        stop=(dc == ND - 1),
                    )
                mt = cpool.tile([128, N], bf16, tag=f"m_T{t}")
                if t < 6:
                    nc.vector.tensor_copy(mt[:], ps[:])
                else:
                    nc.scalar.copy(mt[:], ps[:])
                m_T.append(mt)
                cf = cpool.tile([128, R], f32, tag=f"col_f{t}")
                nc.scalar.copy(cf[:], mt[:, 0:R])
                col_f.append(cf)

            # col_neg[t] = -col_f[t] for the ACT Relu-bias tiles only
            col_n = {}
            for t in sorted({t for par in ACT_SPLIT for (_h, t) in par}):
                cn = cpool.tile([128, R], f32, tag=f"col_n{t}", name=f"col_n{t}")
                nc.scalar.mul(cn[:], col_f[t][:], -1.0)
                col_n[t] = cn

            # ---- Stage C2: Mo_sum[o, j] = sum_h M_T[oh, j] ---------------
            # Accumulate G = P + Mh (Mh = -Mo_sum/2 in bf16) on PE; fold the
            # per-i +Mo_sum[:, i] into the exp bias B = 2*Mh[:, i].
            mo_ps = mmps.tile([O, N], f32, tag="ps")
            for t in range(NT):
                lo = t // 4
                nc.tensor.matmul(mo_ps[32 * lo:32 * lo + 32, :],
                                 mask[:, t, :], m_T[t][:],
                                 start=(t % 4 == 0), stop=(t % 4 == 3),
                                 tile_position=(0, 32 * lo),
                                 skip_group_check=True)
            mh = cpool.tile([O, N], bf16)
            nc.scalar.mul(mh[:], mo_ps[:], -0.5)
            # paired exp bias: rows 0:64 even-i cols, 64:128 odd-i cols
            bexp2 = cpool.tile([2 * O, R // 2], f32)
            nc.scalar.mul(bexp2[0:O, :], mh[:, 0:R:2], 2.0)
            nc.scalar.mul(bexp2[O:2 * O, :], mh[:, 1:R:2], 2.0)

            # ---- Stage D: main loop, two rows per PSUM tile --------------
            # Pair (2k, 2k+1): one [128, N] psum; even row in partitions
            # 0:64, odd in 64:128; p = 64*half + o.  13 of the 16 relu
            # tiles on DVE (dual-op tensor_scalar), 3 on ACT.  The two
            # ident MMs + 16 mask MMs are emitted in col-group quads so
            # four streams run concurrently on the PE.
            LAG = 3
            s_pair = cpool.tile([2 * O, R // 2], f32)
            pend = []

            def emit_exp(ps, k):
                e_scr = epool.tile([2 * O, N], bf16, tag="e_scr")
                nc.scalar.activation(
                    e_scr[:], ps[:], Act.Exp, bias=bexp2[:, k:k + 1], scale=-2.0,
                    accum_out=s_pair[:, k:k + 1],
                )

            for k in range(R // 2):
                act_set = set(ACT_SPLIT[k % 2])
                abigs = []
                for half in range(2):
                    i = 2 * k + half
                    a_big = apool.tile([128, NT, N], bf16, tag="a_big")
                    for t in range(NT):
                        if (half, t) in act_set:
                            nc.scalar.activation(
                                a_big[:, t, :], m_T[t][:], Act.Relu,
                                bias=col_n[t][:, i:i + 1], scale=1.0,
                            )
                        else:
                            nc.vector.tensor_scalar(
                                a_big[:, t, :], m_T[t][:],
                                col_f[t][:, i:i + 1], 0.0,
                                Alu.subtract, Alu.max,
                            )
                    abigs.append(a_big)
                ps = l1ps.tile([2 * O, N], f32, tag="l1")
                for half in range(2):
                    nc.tensor.matmul(
                        ps[half * O:(half + 1) * O, :], ident[:, 0:O], mh[:],
                        start=True, stop=False, tile_position=(0, half * O),
                        skip_group_check=True,
                    )
                for q in range(4):
                    for half in range(2):
                        for lo in range(2):
                            t = q + 4 * lo
                            base = 64 * half + 32 * lo
                            nc.tensor.matmul(
                                ps[base:base + 32, :], mask[:, t, :],
                                abigs[half][:, t, :],
                                start=False, stop=(q == 3),
                                tile_position=(0, base),
                                skip_group_check=True,
                            )
                pend.append((ps, k))
                if len(pend) > LAG:
                    emit_exp(*pend.pop(0))
            for args in pend:
                emit_exp(*args)
            # unpack pairs into S[o, i]
            s_all = cpool.tile([O, R], f32)
            nc.vector.tensor_copy(s_all[:, 0:R:2], s_pair[0:O, :])
            nc.vector.tensor_copy(s_all[:, 1:R:2], s_pair[O:2 * O, :])

            # ---- Stage E: transpose S, subtract 1, write out -------------
            s_T = cpool.tile([R, O], f32)
            for a in range(2):
                for b in range(2):
                    nc.vector.transpose(
                        s_T[32 * a:32 * a + 32, 32 * b:32 * b + 32],
                        s_all[32 * b:32 * b + 32, 32 * a:32 * a + 32],
                    )
            o_small = cpool.tile([R, O], f32)
            nc.vector.tensor_scalar_add(o_small[:], s_T[:], -1.0)
            nc.sync.dma_start(out_d[:, D:D + O], o_small[:])
            nc.sync.dma_start(out_d[:, 0:D], x_f[0][0:R, :])

    nc.compile()
    return nc


def _get_nc():
    if "nc" not in _cache:
        _cache["nc"] = _build()
    return _cache["nc"]


def kernel(x, T):
    from concourse import bass_utils

    nc = _get_nc()
    x = np.ascontiguousarray(x, dtype=np.float32)
    T = np.ascontiguousarray(T, dtype=np.float32)
    mask = _mask32_np()
    ident = np.concatenate([np.eye(O), np.eye(O)], axis=1).astype(ml_dtypes.bfloat16)
    ident128 = np.eye(128, dtype=ml_dtypes.bfloat16)
    in_maps = [
        {"x": np.roll(x, -R * c, axis=0), "T": T, "mask": mask, "ident": ident, "ident128": ident128}
        for c in range(NCORES)
    ]
    res = bass_utils.run_bass_kernel_spmd(nc, in_maps, list(range(NCORES)))
    return np.concatenate([res.results[c]["out"] for c in range(NCORES)], axis=0)
